# revision 1
# baseline (speedup 1.0000x reference)
"""Biaffine scorer kernel for Trainium2 (Bass/Tile), data-parallel over batch
across 8 NeuronCores.

Reference computation (per batch item b):
    h = leaky_relu(state @ head_w + head_b)          # (S, BS)
    t = leaky_relu(state @ tail_w + tail_b)          # (S, BS)
    scores1[x,y,o] = h[x] @ U[o] @ t[y]
    scores2[x,y,o] = Wh·h1[x] + Wt·t1[y] + Ww·wemb[x,y] + cls_b
    out = scores1 + scores2                          # (S, S, O)

Device-side decomposition. Everything is feature-major (contraction dim on
SBUF partitions); S padded 255->256; batch items processed in PAIRS so every
matmul streams N=512 columns (fp32r full rate, LDWEIGHTS fully hidden):

    h1T/t1T [121, 512]  = leaky(head_w.T @ stateT + bias) for (b0|b1).
                          Bias (and the ones-row 1.0) enters as a K=1
                          accumulating matmul: bias_row.T x ones_row.
    tUT [121, 2x2560]   : per o, [U(o).T | Wt(o)] @ t1T -> strided-copied to
                          columns b*2560 + (o::10); row 120 = B[y,o]=Wt.t1[y]
    A_T [10, 512]       = WhT_ext.T @ h1T
    out[x, (y,o)]       = h1T.T @ tUT  (+ A via 0/1-indicator matmul as a 2nd
                          PSUM pass) (+ C table via the DVE add that also
                          evacuates PSUM->SBUF)

C[x, y*10+o] = width_table[pos(x,y)] @ Ww.T + cls_b is precomputed on host
(tiny). The output [x, (y,o)] layout makes output DMAs fully contiguous
128-row blocks (row-multiple-of-16 so the qSP HWDGE ring spreads descriptors
across all 16 SDMA engines); inputs ride the qAct ring.
"""

import os
import numpy as np

import concourse.bass as bass
import concourse.bacc as bacc
import concourse.tile as tile
from concourse import mybir
from concourse.bass_utils import run_bass_kernel_spmd

# problem shape (hardcoded per harness contract)
B, S, H = 32, 255, 1024
BS, WD, O = 120, 20, 10
HSZ = 2 * (BS + 1) + WD
SP = 256            # padded S
SP2 = 2 * SP        # paired moving dim
NW = SP * O         # 2560
NREAL = S * O       # 2550
KT = H // 128       # 8
NCORES = 8
BPC = B // NCORES   # 4 batch items per core
NP = BPC // 2       # 2 pairs per core
BSE = BS + 1        # 121

F32 = mybir.dt.float32
F32R = mybir.dt.float32r

_CACHE: dict = {}


def _emit(tc, d):
    """Emit the per-core program. d: dict of DRAM APs."""
    from contextlib import ExitStack

    nc = tc.nc
    AF = mybir.ActivationFunctionType
    ALU = mybir.AluOpType

    with ExitStack() as ctx:
        const = ctx.enter_context(tc.tile_pool(name="const", bufs=1))
        st_pool = ctx.enter_context(tc.tile_pool(name="st", bufs=2))
        ht_pool = ctx.enter_context(tc.tile_pool(name="ht", bufs=2))
        tut_pool = ctx.enter_context(tc.tile_pool(name="tut", bufs=2))
        out_pool = ctx.enter_context(tc.tile_pool(name="outp", bufs=3))
        pp_ht = ctx.enter_context(tc.tile_pool(name="pp_ht", bufs=1, space="PSUM"))
        pp_u = ctx.enter_context(tc.tile_pool(name="pp_u", bufs=2, space="PSUM"))
        pp_s = ctx.enter_context(tc.tile_pool(name="pp_s", bufs=4, space="PSUM"))

        # ---- persistent constants ----
        # head/tail weights carry an extra zero column (-> psum row 120 = 0);
        # biases + the ones-row 1.0 enter via the K=1 bias matmul below.
        # single-partition row first: [1,512] ones | [1,121] hb | [1,121] tb
        sb_row = const.tile([1, SP2 + 2 * BSE], F32R)
        nc.sync.dma_start(sb_row[:], d["row"])
        sb_hw = const.tile([128, KT * BSE], F32R)
        nc.sync.dma_start(sb_hw[:], d["hw"])
        sb_tw = const.tile([128, KT * BSE], F32R)
        nc.sync.dma_start(sb_tw[:], d["tw"])
        # ut: per-o [121, 121] blocks, then 10 cols of WhT_ext.
        sb_ut = const.tile([BSE, O * BSE + 12], F32R)
        nc.sync.dma_start(sb_ut[:], d["ut"])
        sb_ones = sb_row[:, 0:SP2]
        sb_hbr = sb_row[:, SP2:SP2 + BSE]
        sb_tbr = sb_row[:, SP2 + BSE:SP2 + 2 * BSE]
        sb_c0 = const.tile([128, NW], F32)
        sb_c1 = const.tile([128, NW], F32)

        for p in range(NP):
            # ---- load paired stateT (host-packed [128, (kt, b01, y)]) ----
            # two separate half-tiles so kt<4 projections depend only on
            # the first transfer (128-row HWDGE reads on the qAct ring spread
            # across all 16 SDMA engines).
            half = KT * SP2 // 2
            sb_sTa = st_pool.tile([128, half], F32R)
            sb_sTb = st_pool.tile([128, half], F32R)
            nc.scalar.dma_start(sb_sTa[:], d["stateT"][p][:, 0:half])
            nc.scalar.dma_start(sb_sTb[:], d["stateT"][p][:, half:])

            # ---- head/tail projections -> h1T/t1T [121, 512] ----
            ps_h = pp_ht.tile([BSE, SP2], F32)
            ps_t = pp_ht.tile([BSE, SP2], F32)
            for ps, w, br in ((ps_h, sb_hw, sb_hbr), (ps_t, sb_tw, sb_tbr)):
                nc.tensor.matmul(
                    ps[:], lhsT=br, rhs=sb_ones, start=True, stop=False
                )
                for kt in range(KT):
                    st = sb_sTa if kt < 4 else sb_sTb
                    nc.tensor.matmul(
                        ps[:],
                        lhsT=w[:, kt * BSE:(kt + 1) * BSE],
                        rhs=st[:, (kt % 4) * SP2:(kt % 4 + 1) * SP2],
                        start=False,
                        stop=(kt == KT - 1),
                    )
            h1T = ht_pool.tile([BSE, SP2], F32R)
            t1T = ht_pool.tile([BSE, SP2], F32R)
            # u = psum ; leaky = max(u, 0.01u)
            nc.scalar.activation(h1T[:], ps_h[:], AF.Copy)
            nc.vector.scalar_tensor_tensor(
                h1T[:], h1T[:], 0.01, h1T[:], op0=ALU.mult, op1=ALU.max
            )
            nc.scalar.activation(t1T[:], ps_t[:], AF.Copy)
            nc.vector.scalar_tensor_tensor(
                t1T[:], t1T[:], 0.01, t1T[:], op0=ALU.mult, op1=ALU.max
            )
            if p == 0:
                # C loads are first needed by p0's finals; ride the qSP ring,
                # which is otherwise idle until the first output (~+55us).
                nc.sync.dma_start(sb_c0[:], d["cmat"][0:128, :])
                nc.sync.dma_start(sb_c1[:], d["cmat"][128:256, :])

            # ---- tUT [121, 2*2560]: interleaved per-(o,b) copies ----
            tUT = tut_pool.tile([BSE, 2 * NW], F32R)
            for o in range(O):
                ps_u = pp_u.tile([BSE, SP2], F32)
                nc.tensor.matmul(
                    ps_u[:],
                    lhsT=sb_ut[:, o * BSE:(o + 1) * BSE],
                    rhs=t1T[:],
                    start=True,
                    stop=True,
                )
                # strided scatter: tUT[:, bb*NW + (o::10)] <- ps_u[:, bb*SP:+SP]
                for bb in range(2):
                    src = ps_u[:, bb * SP:(bb + 1) * SP]
                    dst = tUT[:, bb * NW + o:bb * NW + NW:O]
                    nc.scalar.activation(dst, src, AF.Copy)

            # ---- finals: out[x, (y,o)] per (b-in-pair, x-tile) ----
            for bb in range(2):
                for xt in range(2):
                    sb_c = sb_c0 if xt == 0 else sb_c1
                    sb_out = out_pool.tile([128, NW], F32)
                    lo = bb * SP + xt * 128
                    for c in range(5):
                        ps_s = pp_s.tile([128, 512], F32)
                        # single pass: the A-term rides inside tUT (folded
                        # into the ut blocks' ones-row on the host).
                        nc.tensor.matmul(
                            ps_s[:],
                            lhsT=h1T[:, lo:lo + 128],
                            rhs=tUT[:, bb * NW + c * 512:bb * NW + (c + 1) * 512],
                            start=True,
                            stop=True,
                        )
                        oc = sb_out[:, c * 512:(c + 1) * 512]
                        cc = sb_c[:, c * 512:(c + 1) * 512]
                        if p == NP - 1 and c >= 3:
                            # drain-phase offload: DVE is the bottleneck at
                            # the tail; route 2/5 chunks via ACT-copy +
                            # GpSimd in-place add (both idle then).
                            nc.scalar.activation(oc, ps_s[:], AF.Copy)
                            nc.gpsimd.tensor_add(oc, oc, cc)
                        else:
                            nc.vector.tensor_add(oc, ps_s[:], cc)
                    # two 64-row (multiple of 16!) contiguous blocks on the
                    # qSP HWDGE ring -> each spreads across the SDMA engines
                    # and the first can start before the last chunk's add.
                    # Row 255 of the padded output absorbs xt=1's garbage.
                    orow = xt * 128
                    nc.sync.dma_start(
                        d["out"][2 * p + bb, orow:orow + 64, :],
                        sb_out[0:64, :],
                    )
                    nc.sync.dma_start(
                        d["out"][2 * p + bb, orow + 64:orow + 128, :],
                        sb_out[64:128, :],
                    )


def build_nc():
    if "nc" in _CACHE:
        return _CACHE["nc"]
    nc = bacc.Bacc(
        "TRN2", target_bir_lowering=False, debug=False, num_devices=NCORES
    )
    d = {}
    d["stateT"] = nc.dram_tensor(
        "stateT", [NP, 128, KT * SP2], F32R, kind="ExternalInput"
    ).ap()
    d["hw"] = nc.dram_tensor("hw", [128, KT * BSE], F32R, kind="ExternalInput").ap()
    d["tw"] = nc.dram_tensor("tw", [128, KT * BSE], F32R, kind="ExternalInput").ap()
    d["ut"] = nc.dram_tensor(
        "ut", [BSE, O * BSE + 12], F32R, kind="ExternalInput"
    ).ap()
    d["row"] = nc.dram_tensor(
        "row", [1, SP2 + 2 * BSE], F32R, kind="ExternalInput"
    ).ap()
    d["cmat"] = nc.dram_tensor("cmat", [SP, NW], F32, kind="ExternalInput").ap()
    d["out"] = nc.dram_tensor("out", [BPC, SP, NW], F32, kind="ExternalOutput").ap()

    with tile.TileContext(nc) as tc:
        _emit(tc, d)
    nc.compile()
    _CACHE["nc"] = nc
    return nc


def prep_inputs(inputs):
    """Host-side constant packing + state transpose. Returns dict of np arrays
    shared across cores (stateT is full-batch; shard before dispatch)."""
    state = np.asarray(inputs["state"], np.float32)
    head_w = np.asarray(inputs["head_w"], np.float32)
    head_b = np.asarray(inputs["head_b"], np.float32)
    tail_w = np.asarray(inputs["tail_w"], np.float32)
    tail_b = np.asarray(inputs["tail_b"], np.float32)
    U = np.asarray(inputs["U"], np.float32)
    width_table = np.asarray(inputs["width_table"], np.float32)
    cls_w = np.asarray(inputs["cls_w"], np.float32)
    cls_b = np.asarray(inputs["cls_b"], np.float32)

    # stateT paired pack: [B/2, 128, (kt, b01, y)], y zero-padded to 256
    stateT = np.zeros((B, H, SP), np.float32)
    stateT[:, :, :S] = state.transpose(0, 2, 1)
    # [B/2, 2, KT, 128, SP] -> [B/2, 128, KT, 2, SP]
    stateT = stateT.reshape(B // 2, 2, KT, 128, SP).transpose(0, 3, 2, 1, 4)
    stateT = np.ascontiguousarray(stateT.reshape(B // 2, 128, KT * SP2))

    hw_sb = np.zeros((128, KT, BSE), np.float32)
    hw_sb[:, :, :BS] = head_w.reshape(KT, 128, BS).transpose(1, 0, 2)
    hw_sb = np.ascontiguousarray(hw_sb.reshape(128, KT * BSE))
    tw_sb = np.zeros((128, KT, BSE), np.float32)
    tw_sb[:, :, :BS] = tail_w.reshape(KT, 128, BS).transpose(1, 0, 2)
    tw_sb = np.ascontiguousarray(tw_sb.reshape(128, KT * BSE))

    # ut blocks + WhT_ext + 2 spare cols
    ut = np.zeros((BSE, O * BSE + 12), np.float32)
    blocks = ut[:, :O * BSE].reshape(BSE, O, BSE)
    blocks[:BS, :, :BS] = U.transpose(2, 0, 1)           # [j, o, i] = U[o,i,j]
    blocks[:, :, BS] = cls_w[:, BS + 1:2 * (BS + 1)].T   # Wt (incl ones coeff)
    # fold the Wh projection (A-term) into the ones-row of each block:
    # t1T row 120 is all-ones, so adding Wh_ext[o, i] here adds A[x, o]
    # (broadcast over y) to the final scores.
    blocks[BS, :, :] += cls_w[:, :BSE]
    ut = np.ascontiguousarray(ut)

    row = np.zeros((1, SP2 + 2 * BSE), np.float32)
    row[0, :S] = 1.0                                     # b0 ones (y=255 -> 0)
    row[0, SP:SP + S] = 1.0                              # b1 ones
    row[0, SP2:SP2 + BS] = head_b
    row[0, SP2 + BS] = 1.0                               # ones-row constant
    row[0, SP2 + BSE:SP2 + BSE + BS] = tail_b
    row[0, SP2 + BSE + BS] = 1.0

    pos = np.arange(S)[None, :] - np.arange(S)[:, None] + 1
    pos = pos * (pos > 0)
    wproj = width_table @ cls_w[:, 2 * (BS + 1):].T + cls_b   # [256, 10]
    cmat = np.zeros((SP, NW), np.float32)
    cmat[:S, :NREAL] = wproj[pos].reshape(S, NREAL)

    return {
        "stateT": stateT,
        "hw": hw_sb,
        "tw": tw_sb,
        "ut": ut,
        "row": row,
        "cmat": cmat,
    }


def run(inputs, trace=False, trace_kwargs=None):
    nc = build_nc()
    full = prep_inputs(inputs)
    shared = {k: v for k, v in full.items() if k != "stateT"}
    in_maps = []
    for c in range(NCORES):
        m = dict(shared)
        m["stateT"] = np.ascontiguousarray(full["stateT"][c * NP:(c + 1) * NP])
        in_maps.append(m)
    res = run_bass_kernel_spmd(
        nc,
        in_maps,
        core_ids=list(range(NCORES)),
        trace=trace,
        **(trace_kwargs or {}),
    )
    out = np.concatenate([r["out"] for r in res.results], axis=0)
    out = out[:, :S, :NREAL].reshape(B, S, S, O)
    return out, res


def kernel(**inputs):
    out, _ = run(inputs, trace=False)
    return out


if __name__ == "__main__":
    build_nc()
    print("build ok")



# revision 5
# speedup vs baseline: 1.4588x; 1.4588x over previous
"""Biaffine scorer kernel for Trainium2 (Bass/Tile), data-parallel over batch
across 8 NeuronCores.

Reference computation (per batch item b):
    h = leaky_relu(state @ head_w + head_b)          # (S, BS)
    t = leaky_relu(state @ tail_w + tail_b)          # (S, BS)
    scores1[x,y,o] = h[x] @ U[o] @ t[y]
    scores2[x,y,o] = Wh.h1[x] + Wt.t1[y] + Ww.wemb[x,y] + cls_b
    out = scores1 + scores2                          # (S, S, O)

v2: full-bf16 dataflow. tolerance is 2e-2 rel; bf16 end-to-end measures
~1.5e-3, and halving every DMA byte matters because the f32 baseline was
DMA-bandwidth-bound (16 engines ~66% busy).

Device-side decomposition per core (4 batch items, processed in 2 pairs
so matmuls stream 512 moving columns):

    h1T/t1T [121, 512]    = Prelu(head_w.T @ stateT, bias) per (b0|b1);
                            bias + the ones-row enter via the ACT bias AP
                            (no K=1 bias matmul), Prelu alpha=0.01 does the
                            leaky in the same PSUM-evacuating ACT op.
    tUT_bb [121, 2560]    : per o, [U(o).T + folds] @ t1_bb -> contiguous
                            (o,y)-blocks, evacuated by plain ACT/DVE copies
                            (the old (y,o) interleave cost 2.3x on ACT).
    out[x, (o,y)]         = h1T.T @ tUT_bb per 512-col chunk; the C table
                            (+cls_b, +width term) rides the PSUM-evacuating
                            add, split DVE / ACT+GpSimd (GpSimd has no PSUM
                            port, so its chunks take an ACT copy first).

Host side packs constants, transposes state, and un-interleaves the
(o,y)-major bf16 output back to (S, S, O) f32.
"""

import numpy as np
import ml_dtypes

import concourse.bass as bass
import concourse.bacc as bacc
import concourse.tile as tile
from concourse import mybir
from concourse.bass_utils import run_bass_kernel_spmd

# problem shape (hardcoded per harness contract)
B, S, H = 32, 255, 1024
BS, WD, O = 120, 20, 10
SP = 256            # padded S
SP2 = 2 * SP        # paired moving dim
NW = SP * O         # 2560
KT = H // 128       # 8
NCORES = 8
BPC = B // NCORES   # 4 batch items per core
NP = BPC // 2       # 2 pairs per core
BSE = BS + 1        # 121

F32 = mybir.dt.float32
BF16 = mybir.dt.bfloat16
NPBF = ml_dtypes.bfloat16

_CACHE: dict = {}


def _emit(tc, d):
    """Emit the per-core program. d: dict of DRAM APs."""
    from contextlib import ExitStack

    nc = tc.nc
    AF = mybir.ActivationFunctionType

    with ExitStack() as ctx:
        const = ctx.enter_context(tc.tile_pool(name="const", bufs=1))
        st_pool = ctx.enter_context(tc.tile_pool(name="st", bufs=2))
        ht_pool = ctx.enter_context(tc.tile_pool(name="ht", bufs=2))
        tut_pool = ctx.enter_context(tc.tile_pool(name="tut", bufs=2))
        out_pool = ctx.enter_context(tc.tile_pool(name="outp", bufs=3))
        pp_ht = ctx.enter_context(tc.tile_pool(name="pp_ht", bufs=1, space="PSUM"))
        pp_u = ctx.enter_context(tc.tile_pool(name="pp_u", bufs=2, space="PSUM"))
        pp_s = ctx.enter_context(tc.tile_pool(name="pp_s", bufs=4, space="PSUM"))

        # ---- persistent constants ----
        # weights carry an extra zero column -> psum row 120 = 0; the ACT
        # bias AP then sets row 120 to Prelu(0 + 1.0) = 1.0 (the ones row).
        sb_hw = const.tile([128, KT * BSE], BF16)
        nc.sync.dma_start(sb_hw[:], d["hw"])
        sb_tw = const.tile([128, KT * BSE], BF16)
        nc.sync.dma_start(sb_tw[:], d["tw"])
        # ut: per-o [121, 121] blocks (U.T with Wt in col 120, Wh folded
        # into the ones-row), then 2 spare cols.
        sb_ut = const.tile([BSE, O * BSE + 2], BF16)
        nc.sync.dma_start(sb_ut[:], d["ut"])
        # bias: col 0 = head_b (+1.0 at row 120), col 1 = tail_b
        sb_bias = const.tile([BSE, 2], F32)
        nc.sync.dma_start(sb_bias[:], d["bias"])
        sb_c0 = const.tile([128, NW], BF16)
        sb_c1 = const.tile([128, NW], BF16)

        for p in range(NP):
            # ---- load paired stateT (host-packed [128, (kt, b01, y)]) ----
            # two half-tiles so kt<4 projections depend only on the first.
            half = KT * SP2 // 2
            sb_sTa = st_pool.tile([128, half], BF16)
            sb_sTb = st_pool.tile([128, half], BF16)
            nc.scalar.dma_start(sb_sTa[:], d["stateT"][p][:, 0:half])
            nc.scalar.dma_start(sb_sTb[:], d["stateT"][p][:, half:])

            # ---- head/tail projections -> h1T/t1T [121, 512] bf16 ----
            ps_h = pp_ht.tile([BSE, SP2], F32)
            ps_t = pp_ht.tile([BSE, SP2], F32)
            for ps, w in ((ps_h, sb_hw), (ps_t, sb_tw)):
                for kt in range(KT):
                    st = sb_sTa if kt < 4 else sb_sTb
                    nc.tensor.matmul(
                        ps[:],
                        lhsT=w[:, kt * BSE:(kt + 1) * BSE],
                        rhs=st[:, (kt % 4) * SP2:(kt % 4 + 1) * SP2],
                        start=(kt == 0),
                        stop=(kt == KT - 1),
                    )
            h1T = ht_pool.tile([BSE, SP2], BF16)
            t1T = ht_pool.tile([BSE, SP2], BF16)
            nc.scalar.activation(
                h1T[:], ps_h[:], AF.Prelu, bias=sb_bias[:, 0:1], alpha=0.01
            )
            nc.scalar.activation(
                t1T[:], ps_t[:], AF.Prelu, bias=sb_bias[:, 1:2], alpha=0.01
            )
            if p == 0:
                # C loads first needed by p0's finals; qSP ring is idle here.
                nc.sync.dma_start(sb_c0[:], d["cmat"][0:128, :])
                nc.sync.dma_start(sb_c1[:], d["cmat"][128:256, :])

            # ---- tUT_bb [121, 2560] bf16, contiguous (o,y)-blocks ----
            tUT = [
                tut_pool.tile([BSE, NW], BF16, name=f"tUT{p}_{i}")
                for i in range(2)
            ]
            for bb in range(2):
                rhs = t1T[:, bb * SP:(bb + 1) * SP]
                for q in range(5):
                    ps_u = pp_u.tile([BSE, 512], F32)
                    for s in range(2):
                        nc.tensor.matmul(
                            ps_u[:, s * SP:(s + 1) * SP],
                            lhsT=sb_ut[:, (2 * q + s) * BSE:(2 * q + s + 1) * BSE],
                            rhs=rhs,
                            start=True,
                            stop=True,
                        )
                    dst = tUT[bb][:, q * 512:(q + 1) * 512]
                    # evac balance across the whole kernel: ACT ~34 ops,
                    # DVE ~30, GpSimd 12 -> ~19/19/15us busy each.
                    if bb == 1 and q == 4:
                        nc.vector.tensor_copy(dst, ps_u[:])
                    else:
                        nc.scalar.activation(dst, ps_u[:], AF.Copy)

            # ---- finals: out[x, (o,y)] per (b-in-pair, x-tile) ----
            for bb in range(2):
                for xt in range(2):
                    sb_c = sb_c0 if xt == 0 else sb_c1
                    sb_out = out_pool.tile([128, NW], BF16)
                    lo = bb * SP + xt * 128
                    for c in range(5):
                        ps_s = pp_s.tile([128, 512], F32)
                        nc.tensor.matmul(
                            ps_s[:],
                            lhsT=h1T[:, lo:lo + 128],
                            rhs=tUT[bb][:, c * 512:(c + 1) * 512],
                            start=True,
                            stop=True,
                        )
                        oc = sb_out[:, c * 512:(c + 1) * 512]
                        cc = sb_c[:, c * 512:(c + 1) * 512]
                        if c == 4 or (c == 3 and bb == 1):
                            # GpSimd has no PSUM port: ACT evacuates, then
                            # GpSimd adds C in SBUF.
                            nc.scalar.activation(oc, ps_s[:], AF.Copy)
                            nc.gpsimd.tensor_add(oc, oc, cc)
                        else:
                            nc.vector.tensor_add(oc, ps_s[:], cc)
                    # two 64-row (multiple of 16) contiguous DMAs on the qSP
                    # HWDGE ring; first can start before the last chunk's add.
                    orow = xt * 128
                    nc.sync.dma_start(
                        d["out"][2 * p + bb, orow:orow + 64, :],
                        sb_out[0:64, :],
                    )
                    nc.sync.dma_start(
                        d["out"][2 * p + bb, orow + 64:orow + 128, :],
                        sb_out[64:128, :],
                    )


def build_nc():
    if "nc" in _CACHE:
        return _CACHE["nc"]
    nc = bacc.Bacc(
        "TRN2", target_bir_lowering=False, debug=False, num_devices=NCORES
    )
    d = {}
    d["stateT"] = nc.dram_tensor(
        "stateT", [NP, 128, KT * SP2], BF16, kind="ExternalInput"
    ).ap()
    d["hw"] = nc.dram_tensor("hw", [128, KT * BSE], BF16, kind="ExternalInput").ap()
    d["tw"] = nc.dram_tensor("tw", [128, KT * BSE], BF16, kind="ExternalInput").ap()
    d["ut"] = nc.dram_tensor(
        "ut", [BSE, O * BSE + 2], BF16, kind="ExternalInput"
    ).ap()
    d["bias"] = nc.dram_tensor("bias", [BSE, 2], F32, kind="ExternalInput").ap()
    d["cmat"] = nc.dram_tensor("cmat", [SP, NW], BF16, kind="ExternalInput").ap()
    d["out"] = nc.dram_tensor("out", [BPC, SP, NW], BF16, kind="ExternalOutput").ap()

    with tile.TileContext(nc) as tc:
        _emit(tc, d)
    nc.compile()
    _CACHE["nc"] = nc
    return nc


def prep_inputs(inputs):
    """Host-side constant packing + state transpose. Returns dict of np arrays
    shared across cores (stateT is full-batch; shard before dispatch)."""
    state = np.asarray(inputs["state"], np.float32)
    head_w = np.asarray(inputs["head_w"], np.float32)
    head_b = np.asarray(inputs["head_b"], np.float32)
    tail_w = np.asarray(inputs["tail_w"], np.float32)
    tail_b = np.asarray(inputs["tail_b"], np.float32)
    U = np.asarray(inputs["U"], np.float32)
    width_table = np.asarray(inputs["width_table"], np.float32)
    cls_w = np.asarray(inputs["cls_w"], np.float32)
    cls_b = np.asarray(inputs["cls_b"], np.float32)

    # stateT paired pack: [B/2, 128, (kt, b01, y)], y zero-padded to 256
    stateT = np.zeros((B, H, SP), np.float32)
    stateT[:, :, :S] = state.transpose(0, 2, 1)
    stateT = stateT.reshape(B // 2, 2, KT, 128, SP).transpose(0, 3, 2, 1, 4)
    stateT = np.ascontiguousarray(
        stateT.reshape(B // 2, 128, KT * SP2).astype(NPBF)
    )

    hw_sb = np.zeros((128, KT, BSE), np.float32)
    hw_sb[:, :, :BS] = head_w.reshape(KT, 128, BS).transpose(1, 0, 2)
    hw_sb = np.ascontiguousarray(hw_sb.reshape(128, KT * BSE).astype(NPBF))
    tw_sb = np.zeros((128, KT, BSE), np.float32)
    tw_sb[:, :, :BS] = tail_w.reshape(KT, 128, BS).transpose(1, 0, 2)
    tw_sb = np.ascontiguousarray(tw_sb.reshape(128, KT * BSE).astype(NPBF))

    # ut blocks + 2 spare cols
    ut = np.zeros((BSE, O * BSE + 2), np.float32)
    blocks = ut[:, :O * BSE].reshape(BSE, O, BSE)
    blocks[:BS, :, :BS] = U.transpose(2, 0, 1)           # [j, o, i] = U[o,i,j]
    blocks[:, :, BS] = cls_w[:, BS + 1:2 * (BS + 1)].T   # Wt (incl ones coeff)
    # fold the Wh projection (A-term) into the ones-row of each block:
    # t1T row 120 is all-ones, so adding Wh_ext[o, i] here adds A[x, o]
    # (broadcast over y) to the final scores.
    blocks[BS, :, :] += cls_w[:, :BSE]
    ut = np.ascontiguousarray(ut.astype(NPBF))

    bias = np.zeros((BSE, 2), np.float32)
    bias[:BS, 0] = head_b
    bias[BS, 0] = 1.0                                    # ones-row constant
    bias[:BS, 1] = tail_b
    bias[BS, 1] = 1.0

    pos = np.arange(S)[None, :] - np.arange(S)[:, None] + 1
    pos = pos * (pos > 0)
    wproj = width_table @ cls_w[:, 2 * (BS + 1):].T + cls_b   # [256, 10]
    cmat = np.zeros((SP, NW), np.float32)
    # (o, y)-major: C[x, o*256 + y] = wproj[pos(x,y), o]
    cmat[:S, :].reshape(S, O, SP)[:, :, :S] = wproj[pos].transpose(0, 2, 1)
    cmat = np.ascontiguousarray(cmat.astype(NPBF))

    return {
        "stateT": stateT,
        "hw": hw_sb,
        "tw": tw_sb,
        "ut": ut,
        "bias": bias,
        "cmat": cmat,
    }


def run(inputs, trace=False, trace_kwargs=None):
    nc = build_nc()
    full = prep_inputs(inputs)
    shared = {k: v for k, v in full.items() if k != "stateT"}
    in_maps = []
    for c in range(NCORES):
        m = dict(shared)
        m["stateT"] = np.ascontiguousarray(full["stateT"][c * NP:(c + 1) * NP])
        in_maps.append(m)
    res = run_bass_kernel_spmd(
        nc,
        in_maps,
        core_ids=list(range(NCORES)),
        trace=trace,
        **(trace_kwargs or {}),
    )
    out = np.concatenate([r["out"] for r in res.results], axis=0)
    # [B, 256, (o,y)] bf16 -> [B, S, S, O] f32
    out = out.reshape(B, SP, O, SP).astype(np.float32)
    out = np.ascontiguousarray(out[:, :S, :, :S].transpose(0, 1, 3, 2))
    return out, res


def kernel(**inputs):
    out, _ = run(inputs, trace=False)
    return out


if __name__ == "__main__":
    build_nc()
    print("build ok")
